# revision 1
# baseline (speedup 1.0000x reference)
"""ChamferLoss Trainium2 kernel.

Strategy (per core, data-parallel over batch: 16 batches / 8 cores = 2 each):
  pdist[b,i,j] = ||x_i||^2 + ||y_j||^2 - 2 x_i.y_j   (first 3 channels)
  loss = mean_bj(min_i pdist) + mean_bi(min_j pdist)

m = -pdist comes from a single K=13 bf16 augmented matmul (hi/lo split gives
fp32-class accuracy at bf16 PE speed):
  x-side rows: [xh(3), xh(3), xl(3), -rxh, -rxl, -1, -1]
  y-side rows: [Yh(3), Yl(3), Yh(3),  1,    1,  Ryh, Ryl],  Y = 2y, Ry=||y||^2
min -> max flip: rowmax via tensor_scalar+accum, colmax via tensor_tensor max.

CRITICAL environment fact (measured): on this axon execution path every
cross-engine semaphore dependency costs ~30-70 us, while back-to-back work on
one engine runs at full speed.  The kernel is therefore structured to minimise
cross-engine edges, not engine-seconds:
  - one full-PSUM fill per (batch,row-tile): 8 matmuls -> [128,4096] f32,
    then exactly one PE->DVE handoff and one DVE->PE handback (64 fills/core)
  - no ACT cast; DVE reduces straight from PSUM in fp32
  - all prep arithmetic on DVE only; PE transposes build the channel-major
    operands on-chip (no strided DRAM round-trips - those are ms-slow)
  - output is per-partition partial sums [128,4]; the host does the final
    128-way gather-sum (pure unsharding arithmetic)
"""

from contextlib import ExitStack

import numpy as np

import concourse.bass as bass
import concourse.bacc as bacc
import concourse.tile as tile
from concourse import bass_isa, mybir
from concourse.bass_utils import run_bass_kernel_spmd
from concourse.masks import make_identity

F32 = mybir.dt.float32
BF16 = mybir.dt.bfloat16
AX = mybir.AxisListType
OP = mybir.AluOpType

NEG_BIG = -3.0e38

B_FULL = 16
N_FULL = 4096
C_FULL = 6
N_CORES = 8


def build_nc(b_loc=2, n=4096, c_in=6, num_devices=8, reps=1):
    """Per-core program. Inputs x,y: [b_loc, n, c_in] f32; output "partial"
    [128, 2*b_loc] f32 per-partition partial sums of rowmax/colmax of -pdist."""
    NP = 128
    NQ = n // NP                  # row-tiles per batch (32)
    TH = NQ                       # transposes per prep psum fill

    nc = bacc.Bacc(
        "TRN2",
        target_bir_lowering=False,
        debug=False,
        enable_asserts=False,
        num_devices=num_devices,
    )

    x_d = nc.declare_dram_parameter("x", [b_loc, n, c_in], F32, isOutput=False).ap()
    y_d = nc.declare_dram_parameter("y", [b_loc, n, c_in], F32, isOutput=False).ap()
    out_d = nc.declare_dram_parameter(
        "partial", [NP, 2 * b_loc], F32, isOutput=True
    ).ap()

    with tile.TileContext(nc) as tc, ExitStack() as ctx:
        prep = ctx.enter_context(tc.tile_pool(name="prep", bufs=2))
        singles = ctx.enter_context(tc.tile_pool(name="singles", bufs=1))
        psum_pool = ctx.enter_context(tc.tile_pool(name="psum", bufs=1, space="PSUM"))
        smalls = ctx.enter_context(tc.tile_pool(name="smalls", bufs=2))

        ident = singles.tile([NP, NP], BF16, tag="ident", name="ident")
        make_identity(nc, ident)
        ident32 = singles.tile([NP, NP], F32, tag="ident32", name="ident32")
        make_identity(nc, ident32)

        def emit_body():
            chx = [singles.tile([13, n], BF16, tag=f"chx{b}", name=f"chx{b}")
                   for b in range(b_loc)]
            chy = [singles.tile([13, n], BF16, tag=f"chy{b}", name=f"chy{b}")
                   for b in range(b_loc)]

            # ---- prep: aug point-major (DVE only), PE-transpose, DVE evac
            for b in range(b_loc):
                for side in ("x", "y"):
                    src = x_d if side == "x" else y_d
                    xin = prep.tile([NP, NQ, c_in], F32, tag="xin")
                    nc.sync.dma_start(
                        out=xin, in_=src[b].rearrange("(p q) c -> p q c", p=NP)
                    )
                    aug = prep.tile([NP, NQ, 13], BF16, tag="aug")
                    sq = prep.tile([NP, NQ, 3], F32, tag="sq")
                    rt = prep.tile([NP, NQ, 1], F32, tag="rt")
                    ch = xin[:, :, 0:3]
                    nc.vector.tensor_mul(sq, ch, ch)
                    nc.vector.tensor_reduce(rt, sq, axis=AX.X, op=OP.add)
                    if side == "x":
                        # [xh xh xl | -rxh -rxl | -1 -1]
                        nc.vector.tensor_copy(aug[:, :, 0:3], ch)
                        nc.vector.tensor_copy(aug[:, :, 3:6], aug[:, :, 0:3])
                        nc.vector.tensor_sub(aug[:, :, 6:9], ch, aug[:, :, 0:3])
                        nc.vector.tensor_scalar_mul(aug[:, :, 9:10], rt, -1.0)
                        nc.vector.scalar_tensor_tensor(
                            aug[:, :, 10:11], rt, -1.0, aug[:, :, 9:10],
                            OP.mult, OP.subtract,
                        )
                        nc.vector.memset(aug[:, :, 11:13], -1.0)
                    else:
                        # [Yh Yl Yh | 1 1 | ryh ryl],  Y = 2y
                        nc.vector.tensor_scalar_mul(aug[:, :, 0:3], ch, 2.0)
                        nc.vector.scalar_tensor_tensor(
                            aug[:, :, 3:6], ch, 2.0, aug[:, :, 0:3],
                            OP.mult, OP.subtract,
                        )
                        nc.vector.tensor_copy(aug[:, :, 6:9], aug[:, :, 0:3])
                        nc.vector.memset(aug[:, :, 9:11], 1.0)
                        nc.vector.tensor_copy(aug[:, :, 11:12], rt)
                        nc.vector.tensor_sub(aug[:, :, 12:13], rt, aug[:, :, 11:12])

                    # one PSUM fill: 32 transposes, then one DVE evacuation
                    pt = psum_pool.tile([NP, n], BF16, tag="ps")
                    for q in range(TH):
                        nc.tensor.transpose(
                            pt[0:13, q * NP : (q + 1) * NP], aug[:, q, :], ident
                        )
                    dst = chx[b] if side == "x" else chy[b]
                    nc.vector.tensor_copy(dst, pt[0:13, :])

            # ---- accumulators (all DVE-resident) ----
            colacc = [singles.tile([NP, n], F32, tag=f"colacc{b}", name=f"colacc{b}")
                      for b in range(b_loc)]
            for b in range(b_loc):
                nc.vector.memset(colacc[b], NEG_BIG)
            rowpart = [singles.tile([NP, NQ], F32, tag=f"rowpart{b}",
                                    name=f"rowpart{b}") for b in range(b_loc)]
            junk = singles.tile([NP, n], F32, tag="junk", name="junk")

            # ---- main: 64 full-PSUM fills, one PE->DVE->PE round-trip each;
            # ONE psum tile reused across fills (no per-fill TileRelease)
            ps = psum_pool.tile([NP, n], F32, tag="ps", name="ps_main")
            for b in range(b_loc):
                for r in range(NQ):
                    lhsT = chx[b][:, r * NP : (r + 1) * NP]
                    for s in range(n // 512):
                        nc.tensor.matmul(
                            ps[:, s * 512 : (s + 1) * 512],
                            lhsT=lhsT,
                            rhs=chy[b][:, s * 512 : (s + 1) * 512],
                            start=True,
                            stop=True,
                        )
                    nc.vector.tensor_scalar(
                        out=junk,
                        in0=ps,
                        scalar1=NEG_BIG,
                        scalar2=None,
                        op0=OP.max,
                        op1=OP.max,
                        accum_out=rowpart[b][:, r : r + 1],
                    )
                    nc.vector.tensor_tensor(colacc[b], colacc[b], ps, op=OP.max)

            # ---- finals ----
            sums = singles.tile([NP, 2 * b_loc], F32, tag="sums", name="sums")
            for b in range(b_loc):
                # row side: max over the two half-row partials, then sum
                nc.vector.tensor_reduce(sums[:, b : b + 1], rowpart[b],
                                        axis=AX.X, op=OP.add)
                # col side: transpose colacc, rowmax-reduce, sum
                cmax = smalls.tile([NP, NQ], F32, tag="cmax")
                for t in range(NQ):
                    nc.tensor.transpose(
                        ps[:, t * NP : (t + 1) * NP],
                        colacc[b][:, t * NP : (t + 1) * NP],
                        ident32,
                    )
                nc.vector.tensor_reduce(
                    cmax, ps.rearrange("p (t v) -> p t v", t=NQ),
                    axis=AX.X, op=OP.max,
                )
                nc.vector.tensor_reduce(sums[:, b_loc + b : b_loc + b + 1], cmax,
                                        axis=AX.X, op=OP.add)
            nc.sync.dma_start(out=out_d, in_=sums)

        for _ in range(reps):
            emit_body()

    nc.compile()
    return nc


_CACHE = {}


def _get_nc():
    if "nc" not in _CACHE:
        _CACHE["nc"] = build_nc(
            b_loc=B_FULL // N_CORES, n=N_FULL, c_in=C_FULL, num_devices=N_CORES
        )
    return _CACHE["nc"]


def kernel(x: np.ndarray, y: np.ndarray) -> np.ndarray:
    x = np.ascontiguousarray(np.asarray(x, dtype=np.float32))
    y = np.ascontiguousarray(np.asarray(y, dtype=np.float32))
    assert x.shape == (B_FULL, N_FULL, C_FULL), x.shape
    nc = _get_nc()
    bl = B_FULL // N_CORES
    in_maps = [
        {
            "x": np.ascontiguousarray(x[i * bl : (i + 1) * bl]),
            "y": np.ascontiguousarray(y[i * bl : (i + 1) * bl]),
        }
        for i in range(N_CORES)
    ]
    res = run_bass_kernel_spmd(nc, in_maps, list(range(N_CORES)))
    total = sum(float(r["partial"].astype(np.float64).sum()) for r in res.results)
    loss = -total / float(B_FULL * N_FULL)
    return np.float32(loss)



# revision 3
# speedup vs baseline: 113.9389x; 113.9389x over previous
"""ChamferLoss Trainium2 kernel.

Strategy (per core, data-parallel over batch: 16 batches / 8 cores = 2 each):
  pdist[b,i,j] = ||x_i||^2 + ||y_j||^2 - 2 x_i.y_j   (first 3 channels)
  loss = mean_bj(min_i pdist) + mean_bi(min_j pdist)

m = -pdist comes from a single K=13 bf16 augmented matmul (hi/lo split gives
fp32-class accuracy at bf16 PE speed):
  x-side rows: [xh(3), xh(3), xl(3), -rxh, -rxl, -1, -1]
  y-side rows: [Yh(3), Yl(3), Yh(3),  1,    1,  Ryh, Ryl],  Y = 2y, Ry=||y||^2
min -> max flip: rowmax via tensor_scalar+accum, colmax via tensor_tensor max.

Main loop is a 3-engine pipeline over half-row-tiles [128, 2048]:
  PE:  4 matmuls fill one PSUM half (4 banks, f32); two halves double-buffer
  ACT: evacuates PSUM f32 -> SBUF bf16 (activation Copy with dtype cast)
  DVE: rowmax via tensor_scalar (bf16 SBUF = 4x mode) + colmax via
       tensor_tensor max into a bf16 accumulator (2x mode)
This keeps all three engines ~busy; ACT evacuation (~1.9us/half-tile) is the
steady-state bottleneck => ~240us main loop + prep/finals.

Finals: per-tile rowmax halves get max-combined then summed; colacc gets
PE-transposed (bf16) and max/add-reduced. Output is per-partition partial
sums [128, 2*b_loc]; the host does the final 128-way gather-sum.

build_nc(reps=R, hw_loop=True) wraps the whole body in a tc.For_i hardware
loop: the NEFF size is then independent of R, which lets a timing harness
measure marginal device time per repetition without NEFF-size-dependent
host/dispatch overhead polluting the difference.
"""

from contextlib import ExitStack

import numpy as np

import concourse.bass as bass
import concourse.bacc as bacc
import concourse.tile as tile
from concourse import bass_isa, mybir
from concourse.bass_utils import run_bass_kernel_spmd
from concourse.masks import make_identity

F32 = mybir.dt.float32
BF16 = mybir.dt.bfloat16
AX = mybir.AxisListType
OP = mybir.AluOpType
ACTF = mybir.ActivationFunctionType

NEG_BIG = -3.0e38

B_FULL = 16
N_FULL = 4096
C_FULL = 6
N_CORES = 8


def build_nc(b_loc=2, n=4096, c_in=6, num_devices=8, reps=1, hw_loop=False):
    """Per-core program. Inputs x,y: [b_loc, n, c_in] f32; output "partial"
    [128, 2*b_loc] f32 per-partition partial sums of rowmax/colmax of -pdist."""
    NP = 128
    NQ = n // NP                  # row-tiles per batch (32)
    NH = 2                        # half-tiles per row-tile
    HW = n // NH                  # half-tile width (2048)

    nc = bacc.Bacc(
        "TRN2",
        target_bir_lowering=False,
        debug=False,
        enable_asserts=False,
        num_devices=num_devices,
    )

    x_d = nc.declare_dram_parameter("x", [b_loc, n, c_in], F32, isOutput=False).ap()
    y_d = nc.declare_dram_parameter("y", [b_loc, n, c_in], F32, isOutput=False).ap()
    out_d = nc.declare_dram_parameter(
        "partial", [NP, 2 * b_loc], F32, isOutput=True
    ).ap()

    with tile.TileContext(nc) as tc, ExitStack() as ctx:
        prep = ctx.enter_context(tc.tile_pool(name="prep", bufs=2))
        singles = ctx.enter_context(tc.tile_pool(name="singles", bufs=1))
        psum_pool = ctx.enter_context(tc.tile_pool(name="psum", bufs=2, space="PSUM"))
        evac = ctx.enter_context(tc.tile_pool(name="evac", bufs=3))
        smalls = ctx.enter_context(tc.tile_pool(name="smalls", bufs=2))

        ident = singles.tile([NP, NP], BF16, tag="ident", name="ident")
        make_identity(nc, ident)

        def emit_body():
            chx = [singles.tile([13, n], BF16, tag=f"chx{b}", name=f"chx{b}")
                   for b in range(b_loc)]
            chy = [singles.tile([13, n], BF16, tag=f"chy{b}", name=f"chy{b}")
                   for b in range(b_loc)]

            # ---- prep: aug point-major (DVE only), PE-transpose, DVE evac
            for b in range(b_loc):
                for side in ("x", "y"):
                    src = x_d if side == "x" else y_d
                    xin = prep.tile([NP, NQ, c_in], F32, tag="xin")
                    nc.sync.dma_start(
                        out=xin, in_=src[b].rearrange("(p q) c -> p q c", p=NP)
                    )
                    aug = prep.tile([NP, NQ, 13], BF16, tag="aug")
                    sq = prep.tile([NP, NQ, 3], F32, tag="sq")
                    rt = prep.tile([NP, NQ, 1], F32, tag="rt")
                    ch = xin[:, :, 0:3]
                    nc.vector.tensor_mul(sq, ch, ch)
                    nc.vector.tensor_reduce(rt, sq, axis=AX.X, op=OP.add)
                    if side == "x":
                        # [xh xh xl | -rxh -rxl | -1 -1]
                        nc.vector.tensor_copy(aug[:, :, 0:3], ch)
                        nc.vector.tensor_copy(aug[:, :, 3:6], aug[:, :, 0:3])
                        nc.vector.tensor_sub(aug[:, :, 6:9], ch, aug[:, :, 0:3])
                        nc.vector.tensor_scalar_mul(aug[:, :, 9:10], rt, -1.0)
                        nc.vector.scalar_tensor_tensor(
                            aug[:, :, 10:11], rt, -1.0, aug[:, :, 9:10],
                            OP.mult, OP.subtract,
                        )
                        nc.vector.memset(aug[:, :, 11:13], -1.0)
                    else:
                        # [Yh Yl Yh | 1 1 | ryh ryl],  Y = 2y
                        nc.vector.tensor_scalar_mul(aug[:, :, 0:3], ch, 2.0)
                        nc.vector.scalar_tensor_tensor(
                            aug[:, :, 3:6], ch, 2.0, aug[:, :, 0:3],
                            OP.mult, OP.subtract,
                        )
                        nc.vector.tensor_copy(aug[:, :, 6:9], aug[:, :, 0:3])
                        nc.vector.memset(aug[:, :, 9:11], 1.0)
                        nc.vector.tensor_copy(aug[:, :, 11:12], rt)
                        nc.vector.tensor_sub(aug[:, :, 12:13], rt, aug[:, :, 11:12])

                    # one PSUM fill: 32 transposes, then one DVE evacuation
                    pt = psum_pool.tile([NP, n], BF16, tag="ps")
                    for q in range(NQ):
                        nc.tensor.transpose(
                            pt[0:13, q * NP : (q + 1) * NP], aug[:, q, :], ident
                        )
                    dst = chx[b] if side == "x" else chy[b]
                    nc.vector.tensor_copy(dst, pt[0:13, :])

            # ---- accumulators ----
            colacc = [singles.tile([NP, n], BF16, tag=f"colacc{b}",
                                   name=f"colacc{b}") for b in range(b_loc)]
            for b in range(b_loc):
                nc.vector.memset(colacc[b], NEG_BIG)
            rowhalf = [singles.tile([NP, NH, NQ], F32, tag=f"rowhalf{b}",
                                    name=f"rowhalf{b}") for b in range(b_loc)]
            junk = singles.tile([NP, HW], BF16, tag="junk", name="junk")

            # ---- main: PE fill -> ACT evac/cast -> DVE rowmax+colmax ----
            for b in range(b_loc):
                for r in range(NQ):
                    lhsT = chx[b][:, r * NP : (r + 1) * NP]
                    for h in range(NH):
                        ps = psum_pool.tile([NP, HW], F32, tag="ps")
                        for s in range(HW // 512):
                            c0 = h * HW + s * 512
                            nc.tensor.matmul(
                                ps[:, s * 512 : (s + 1) * 512],
                                lhsT=lhsT,
                                rhs=chy[b][:, c0 : c0 + 512],
                                start=True,
                                stop=True,
                            )
                        sb = evac.tile([NP, HW], BF16, tag="sb")
                        nc.scalar.activation(sb, ps, ACTF.Copy)
                        nc.vector.tensor_scalar(
                            out=junk,
                            in0=sb,
                            scalar1=NEG_BIG,
                            scalar2=None,
                            op0=OP.max,
                            op1=OP.max,
                            accum_out=rowhalf[b][:, h, r : r + 1],
                        )
                        nc.vector.tensor_tensor(
                            colacc[b][:, h * HW : (h + 1) * HW],
                            colacc[b][:, h * HW : (h + 1) * HW],
                            sb,
                            op=OP.max,
                        )

            # ---- finals ----
            sums = singles.tile([NP, 2 * b_loc], F32, tag="sums", name="sums")
            for b in range(b_loc):
                # row side: max over the two half-tile partials, then sum
                rmax = smalls.tile([NP, NQ], F32, tag="rmax")
                nc.vector.tensor_tensor(
                    rmax, rowhalf[b][:, 0, :], rowhalf[b][:, 1, :], op=OP.max
                )
                nc.vector.tensor_reduce(sums[:, b : b + 1], rmax,
                                        axis=AX.X, op=OP.add)
                # col side: transpose colacc (bf16), rowmax-reduce, sum
                pt2 = psum_pool.tile([NP, n], BF16, tag="ps")
                for t in range(NQ):
                    nc.tensor.transpose(
                        pt2[:, t * NP : (t + 1) * NP],
                        colacc[b][:, t * NP : (t + 1) * NP],
                        ident,
                    )
                cmax = smalls.tile([NP, NQ], F32, tag="cmax")
                nc.vector.tensor_reduce(
                    cmax, pt2.rearrange("p (t v) -> p t v", t=NQ),
                    axis=AX.X, op=OP.max,
                )
                nc.vector.tensor_reduce(sums[:, b_loc + b : b_loc + b + 1], cmax,
                                        axis=AX.X, op=OP.add)
            nc.sync.dma_start(out=out_d, in_=sums)

        if hw_loop:
            with tc.For_i(0, reps, 1):
                emit_body()
        else:
            for _ in range(reps):
                emit_body()

    nc.compile()
    return nc


_CACHE = {}


def _get_nc():
    if "nc" not in _CACHE:
        _CACHE["nc"] = build_nc(
            b_loc=B_FULL // N_CORES, n=N_FULL, c_in=C_FULL, num_devices=N_CORES
        )
    return _CACHE["nc"]


def kernel(x: np.ndarray, y: np.ndarray) -> np.ndarray:
    x = np.ascontiguousarray(np.asarray(x, dtype=np.float32))
    y = np.ascontiguousarray(np.asarray(y, dtype=np.float32))
    assert x.shape == (B_FULL, N_FULL, C_FULL), x.shape
    nc = _get_nc()
    bl = B_FULL // N_CORES
    in_maps = [
        {
            "x": np.ascontiguousarray(x[i * bl : (i + 1) * bl]),
            "y": np.ascontiguousarray(y[i * bl : (i + 1) * bl]),
        }
        for i in range(N_CORES)
    ]
    res = run_bass_kernel_spmd(nc, in_maps, list(range(N_CORES)))
    total = sum(float(r["partial"].astype(np.float64).sum()) for r in res.results)
    loss = -total / float(B_FULL * N_FULL)
    return np.float32(loss)


# revision 6
# speedup vs baseline: 117.0399x; 1.0272x over previous
"""ChamferLoss Trainium2 kernel.

Strategy (per core, data-parallel over batch: 16 batches / 8 cores = 2 each):
  pdist[b,i,j] = ||x_i||^2 + ||y_j||^2 - 2 x_i.y_j   (first 3 channels)
  loss = mean_bj(min_i pdist) + mean_bi(min_j pdist)

m = -pdist comes from a single K=13 bf16 augmented matmul (hi/lo split gives
fp32-class accuracy at bf16 PE speed):
  x-side rows: [xh(3), xh(3), xl(3), -rxh, -rxl, -1, -1]
  y-side rows: [Yh(3), Yl(3), Yh(3),  1,    1,  Ryh, Ryl],  Y = 2y, Ry=||y||^2
min -> max flip: rowmax via tensor_scalar+accum, colmax via tensor_tensor max.

Main loop is a 3-engine pipeline over half-row-tiles [128, 2048]:
  PE:  4 matmuls fill one PSUM half (4 banks, f32); two halves double-buffer
  ACT: evacuates PSUM f32 -> SBUF bf16 (activation Copy with dtype cast)
  DVE: rowmax via tensor_scalar (bf16 SBUF = 4x mode) + colmax via
       tensor_tensor max into a bf16 accumulator (2x mode)
This keeps all three engines ~busy; ACT evacuation (~1.9us/half-tile) is the
steady-state bottleneck => ~240us main loop + prep/finals.

Finals: per-tile rowmax halves get max-combined then summed; colacc gets
PE-transposed (bf16) and max/add-reduced. Output is per-partition partial
sums [128, 2*b_loc]; the host does the final 128-way gather-sum.

build_nc(reps=R, hw_loop=True) wraps the whole body in a tc.For_i hardware
loop: the NEFF size is then independent of R, which lets a timing harness
measure marginal device time per repetition without NEFF-size-dependent
host/dispatch overhead polluting the difference.
"""

from contextlib import ExitStack

import numpy as np

import concourse.bass as bass
import concourse.bacc as bacc
import concourse.tile as tile
from concourse import bass_isa, mybir
from concourse.bass_utils import run_bass_kernel_spmd
from concourse.masks import make_identity

F32 = mybir.dt.float32
BF16 = mybir.dt.bfloat16
AX = mybir.AxisListType
OP = mybir.AluOpType
ACTF = mybir.ActivationFunctionType

NEG_BIG = -3.0e38

B_FULL = 16
N_FULL = 4096
C_FULL = 6
N_CORES = 8


def build_nc(b_loc=2, n=4096, c_in=6, num_devices=8, reps=1, hw_loop=False):
    """Per-core program. Inputs x,y: [b_loc, n, c_in] f32; output "partial"
    [128, 2*b_loc] f32 per-partition partial sums of rowmax/colmax of -pdist."""
    NP = 128
    NQ = n // NP                  # row-tiles per batch (32)
    NH = 2                        # half-tiles per row-tile
    HW = n // NH                  # half-tile width (2048)

    nc = bacc.Bacc(
        "TRN2",
        target_bir_lowering=False,
        debug=False,
        enable_asserts=False,
        num_devices=num_devices,
    )

    x_d = nc.declare_dram_parameter("x", [b_loc, n, c_in], F32, isOutput=False).ap()
    y_d = nc.declare_dram_parameter("y", [b_loc, n, c_in], F32, isOutput=False).ap()
    out_d = nc.declare_dram_parameter(
        "partial", [NP, 2 * b_loc], F32, isOutput=True
    ).ap()

    with tile.TileContext(nc) as tc, ExitStack() as ctx:
        prep = ctx.enter_context(tc.tile_pool(name="prep", bufs=2))
        singles = ctx.enter_context(tc.tile_pool(name="singles", bufs=1))
        psum_pool = ctx.enter_context(tc.tile_pool(name="psum", bufs=2, space="PSUM"))
        evac = ctx.enter_context(tc.tile_pool(name="evac", bufs=3))
        smalls = ctx.enter_context(tc.tile_pool(name="smalls", bufs=2))

        ident = singles.tile([NP, NP], BF16, tag="ident", name="ident")
        make_identity(nc, ident)

        def emit_body():
            chx = [singles.tile([13, n], BF16, tag=f"chx{b}", name=f"chx{b}")
                   for b in range(b_loc)]
            chy = [singles.tile([13, n], BF16, tag=f"chy{b}", name=f"chy{b}")
                   for b in range(b_loc)]

            # ---- prep: aug point-major (DVE only), PE-transpose, DVE evac
            for b in range(b_loc):
                for side in ("x", "y"):
                    src = x_d if side == "x" else y_d
                    xin = prep.tile([NP, NQ, c_in], F32, tag="xin")
                    nc.sync.dma_start(
                        out=xin, in_=src[b].rearrange("(p q) c -> p q c", p=NP)
                    )
                    aug = prep.tile([NP, NQ, 13], BF16, tag="aug")
                    sq = prep.tile([NP, NQ, 3], F32, tag="sq")
                    rt = prep.tile([NP, NQ, 1], F32, tag="rt")
                    ch = xin[:, :, 0:3]
                    nc.vector.tensor_mul(sq, ch, ch)
                    nc.vector.tensor_reduce(rt, sq, axis=AX.X, op=OP.add)
                    if side == "x":
                        # [xh xh xl | -rxh -rxl | -1 -1]
                        nc.vector.tensor_copy(aug[:, :, 0:3], ch)
                        nc.vector.tensor_copy(aug[:, :, 3:6], aug[:, :, 0:3])
                        nc.vector.tensor_sub(aug[:, :, 6:9], ch, aug[:, :, 0:3])
                        nc.vector.tensor_scalar_mul(aug[:, :, 9:10], rt, -1.0)
                        nc.vector.scalar_tensor_tensor(
                            aug[:, :, 10:11], rt, -1.0, aug[:, :, 9:10],
                            OP.mult, OP.subtract,
                        )
                        nc.vector.memset(aug[:, :, 11:13], -1.0)
                    else:
                        # [Yh Yl Yh | 1 1 | ryh ryl],  Y = 2y
                        nc.vector.tensor_scalar_mul(aug[:, :, 0:3], ch, 2.0)
                        nc.vector.scalar_tensor_tensor(
                            aug[:, :, 3:6], ch, 2.0, aug[:, :, 0:3],
                            OP.mult, OP.subtract,
                        )
                        nc.vector.tensor_copy(aug[:, :, 6:9], aug[:, :, 0:3])
                        nc.vector.memset(aug[:, :, 9:11], 1.0)
                        nc.vector.tensor_copy(aug[:, :, 11:12], rt)
                        nc.vector.tensor_sub(aug[:, :, 12:13], rt, aug[:, :, 11:12])

                    # one PSUM fill: 32 transposes, then one DVE evacuation
                    pt = psum_pool.tile([NP, n], BF16, tag="ps")
                    for q in range(NQ):
                        nc.tensor.transpose(
                            pt[0:13, q * NP : (q + 1) * NP], aug[:, q, :], ident
                        )
                    dst = chx[b] if side == "x" else chy[b]
                    nc.vector.tensor_copy(dst, pt[0:13, :])

            # ---- accumulators ----
            colacc = [singles.tile([NP, n], BF16, tag=f"colacc{b}",
                                   name=f"colacc{b}") for b in range(b_loc)]
            for b in range(b_loc):
                nc.vector.memset(colacc[b], NEG_BIG)
            rowtile = [singles.tile([NP, NQ], F32, tag=f"rowtile{b}",
                                    name=f"rowtile{b}") for b in range(b_loc)]

            # ---- main: PE fill -> ACT evac/cast -> DVE colmax TT + rowmax
            # fold-tree.  All DVE reduce ops are 1x mode, but TENSOR_TENSOR
            # max on bf16 SBUF runs 2x, so the per-tile rowmax is computed as
            # a pairwise-max fold (4096->1024->512->256) followed by one
            # small 1x cache-reduce: ~1.3us/half-tile instead of 2.3us.
            for b in range(b_loc):
                for r in range(NQ):
                    lhsT = chx[b][:, r * NP : (r + 1) * NP]
                    tt = []
                    for h in range(NH):
                        ps = psum_pool.tile([NP, HW], F32, tag="ps")
                        for s in range(HW // 512):
                            c0 = h * HW + s * 512
                            nc.tensor.matmul(
                                ps[:, s * 512 : (s + 1) * 512],
                                lhsT=lhsT,
                                rhs=chy[b][:, c0 : c0 + 512],
                                start=True,
                                stop=True,
                            )
                        sb = evac.tile([NP, HW], BF16, tag="sb")
                        nc.scalar.activation(sb, ps, ACTF.Copy)
                        nc.vector.tensor_tensor(
                            colacc[b][:, h * HW : (h + 1) * HW],
                            colacc[b][:, h * HW : (h + 1) * HW],
                            sb,
                            op=OP.max,
                        )
                        t = smalls.tile([NP, HW // 2], BF16, tag="t")
                        nc.vector.tensor_tensor(
                            t, sb[:, 0 : HW // 2], sb[:, HW // 2 : HW], op=OP.max
                        )
                        tt.append(t)
                    u = smalls.tile([NP, HW // 2], BF16, tag="u")
                    nc.vector.tensor_tensor(u, tt[0], tt[1], op=OP.max)
                    v = smalls.tile([NP, HW // 4], BF16, tag="v")
                    nc.vector.tensor_tensor(
                        v, u[:, 0 : HW // 4], u[:, HW // 4 : HW // 2], op=OP.max
                    )
                    w = smalls.tile([NP, HW // 8], BF16, tag="w")
                    nc.vector.tensor_tensor(
                        w, v[:, 0 : HW // 8], v[:, HW // 8 : HW // 4], op=OP.max
                    )
                    junkw = smalls.tile([NP, HW // 8], BF16, tag="junkw")
                    nc.vector.tensor_scalar(
                        out=junkw,
                        in0=w,
                        scalar1=NEG_BIG,
                        scalar2=None,
                        op0=OP.max,
                        op1=OP.max,
                        accum_out=rowtile[b][:, r : r + 1],
                    )

            # ---- finals ----
            sums = singles.tile([NP, 2 * b_loc], F32, tag="sums", name="sums")
            for b in range(b_loc):
                # row side: sum the per-tile rowmax partials
                nc.vector.tensor_reduce(sums[:, b : b + 1], rowtile[b],
                                        axis=AX.X, op=OP.add)
                # col side: transpose colacc (bf16), rowmax-reduce, sum
                pt2 = psum_pool.tile([NP, n], BF16, tag="ps")
                for t in range(NQ):
                    nc.tensor.transpose(
                        pt2[:, t * NP : (t + 1) * NP],
                        colacc[b][:, t * NP : (t + 1) * NP],
                        ident,
                    )
                cmax = smalls.tile([NP, NQ], F32, tag="cmax")
                nc.vector.tensor_reduce(
                    cmax, pt2.rearrange("p (t v) -> p t v", t=NQ),
                    axis=AX.X, op=OP.max,
                )
                nc.vector.tensor_reduce(sums[:, b_loc + b : b_loc + b + 1], cmax,
                                        axis=AX.X, op=OP.add)
            nc.sync.dma_start(out=out_d, in_=sums)

        if hw_loop:
            with tc.For_i(0, reps, 1):
                emit_body()
        else:
            for _ in range(reps):
                emit_body()

    nc.compile()
    return nc


_CACHE = {}


def _get_nc():
    if "nc" not in _CACHE:
        _CACHE["nc"] = build_nc(
            b_loc=B_FULL // N_CORES, n=N_FULL, c_in=C_FULL, num_devices=N_CORES
        )
    return _CACHE["nc"]


def kernel(x: np.ndarray, y: np.ndarray) -> np.ndarray:
    x = np.ascontiguousarray(np.asarray(x, dtype=np.float32))
    y = np.ascontiguousarray(np.asarray(y, dtype=np.float32))
    assert x.shape == (B_FULL, N_FULL, C_FULL), x.shape
    nc = _get_nc()
    bl = B_FULL // N_CORES
    in_maps = [
        {
            "x": np.ascontiguousarray(x[i * bl : (i + 1) * bl]),
            "y": np.ascontiguousarray(y[i * bl : (i + 1) * bl]),
        }
        for i in range(N_CORES)
    ]
    res = run_bass_kernel_spmd(nc, in_maps, list(range(N_CORES)))
    total = sum(float(r["partial"].astype(np.float64).sum()) for r in res.results)
    loss = -total / float(B_FULL * N_FULL)
    return np.float32(loss)


# revision 14
# speedup vs baseline: 151.0652x; 1.2907x over previous
"""ChamferLoss Trainium2 kernel.

Strategy (per core, data-parallel over batch: 16 batches / 8 cores = 2 each):
  pdist[b,i,j] = ||x_i||^2 + ||y_j||^2 - 2 x_i.y_j   (first 3 channels)
  loss = mean_bj(min_i pdist) + mean_bi(min_j pdist)

m = -pdist comes from a single K=13 bf16 augmented matmul (hi/lo split gives
fp32-class accuracy at bf16 PE speed):
  x-side rows: [xh(3), xh(3), xl(3), -rxh, -rxl, -1, -1]
  y-side rows: [Yh(3), Yl(3), Yh(3),  1,    1,  Ryh, Ryl],  Y = 2y, Ry=||y||^2
min -> max flip: rowmax via tensor_scalar+accum, colmax via tensor_tensor max.

Main loop is a 3-engine pipeline over half-row-tiles [128, 2048]:
  PE:  4 matmuls fill one PSUM half (4 banks, f32); two halves double-buffer
  ACT: evacuates PSUM f32 -> SBUF bf16 (activation Copy with dtype cast)
  DVE: rowmax via tensor_scalar (bf16 SBUF = 4x mode) + colmax via
       tensor_tensor max into a bf16 accumulator (2x mode)
This keeps all three engines ~busy; ACT evacuation (~1.9us/half-tile) is the
steady-state bottleneck => ~240us main loop + prep/finals.

Finals: per-tile rowmax halves get max-combined then summed; colacc gets
PE-transposed (bf16) and max/add-reduced. Output is per-partition partial
sums [128, 2*b_loc]; the host does the final 128-way gather-sum.

build_nc(reps=R, hw_loop=True) wraps the whole body in a tc.For_i hardware
loop: the NEFF size is then independent of R, which lets a timing harness
measure marginal device time per repetition without NEFF-size-dependent
host/dispatch overhead polluting the difference.
"""

from contextlib import ExitStack

import numpy as np

import concourse.bass as bass
import concourse.bacc as bacc
import concourse.tile as tile
from concourse import bass_isa, mybir
from concourse.bass_utils import run_bass_kernel_spmd
from concourse.masks import make_identity

F32 = mybir.dt.float32
BF16 = mybir.dt.bfloat16
AX = mybir.AxisListType
OP = mybir.AluOpType
ACTF = mybir.ActivationFunctionType

NEG_BIG = -3.0e38

B_FULL = 16
N_FULL = 4096
C_FULL = 6
N_CORES = 8


def build_nc(b_loc=2, n=4096, c_in=6, num_devices=8, reps=1, hw_loop=False,
             staggered_reset=True):
    """Per-core program. Inputs x,y: [b_loc, n, c_in] f32; output "partial"
    [128, 2*b_loc] f32 per-partition partial sums of rowmax/colmax of -pdist."""
    NP = 128
    NQ = n // NP                  # row-tiles per batch (32)
    NH = 2                        # half-tiles per row-tile
    HW = n // NH                  # half-tile width (2048)

    nc = bacc.Bacc(
        "TRN2",
        target_bir_lowering=False,
        debug=False,
        enable_asserts=False,
        num_devices=num_devices,
    )

    x_d = nc.declare_dram_parameter("x", [b_loc, n, c_in], F32, isOutput=False).ap()
    y_d = nc.declare_dram_parameter("y", [b_loc, n, c_in], F32, isOutput=False).ap()
    out_d = nc.declare_dram_parameter(
        "partial", [NP, 2 * b_loc], F32, isOutput=True
    ).ap()

    with tile.TileContext(nc) as tc, ExitStack() as ctx:
        prep = ctx.enter_context(tc.tile_pool(name="prep", bufs=2))
        singles = ctx.enter_context(tc.tile_pool(name="singles", bufs=1))
        psum_pool = ctx.enter_context(tc.tile_pool(name="psum", bufs=2, space="PSUM"))
        evac = ctx.enter_context(tc.tile_pool(name="evac", bufs=3))
        smalls = ctx.enter_context(tc.tile_pool(name="smalls", bufs=2))

        ident = singles.tile([NP, NP], BF16, tag="ident", name="ident")
        make_identity(nc, ident)

        def emit_body():
            chx = [singles.tile([13, n], BF16, tag=f"chx{b}", name=f"chx{b}")
                   for b in range(b_loc)]
            chy = [singles.tile([13, n], BF16, tag=f"chy{b}", name=f"chy{b}")
                   for b in range(b_loc)]

            # ---- prep: aug point-major (DVE only), PE-transpose, DVE evac
            for b in range(b_loc):
                for side in ("x", "y"):
                    src = x_d if side == "x" else y_d
                    xin = prep.tile([NP, NQ, c_in], F32, tag="xin")
                    nc.sync.dma_start(
                        out=xin, in_=src[b].rearrange("(p q) c -> p q c", p=NP)
                    )
                    # channel dim padded 13 -> 32 so 4 q's pack into one
                    # [128,128] transpose with 32-aligned output partitions
                    aug = prep.tile([NP, NQ, 32], BF16, tag="aug")
                    sq = prep.tile([NP, NQ, 3], F32, tag="sq")
                    rt = prep.tile([NP, NQ, 1], F32, tag="rt")
                    ch = xin[:, :, 0:3]
                    nc.vector.memset(aug[:, :, 13:32], 0.0)
                    nc.vector.tensor_mul(sq, ch, ch)
                    nc.vector.tensor_reduce(rt, sq, axis=AX.X, op=OP.add)
                    if side == "x":
                        # [xh xh xl | -rxh -rxl | -1 -1]
                        nc.vector.tensor_copy(aug[:, :, 0:3], ch)
                        nc.vector.tensor_copy(aug[:, :, 3:6], aug[:, :, 0:3])
                        nc.vector.tensor_sub(aug[:, :, 6:9], ch, aug[:, :, 0:3])
                        nc.vector.tensor_scalar_mul(aug[:, :, 9:10], rt, -1.0)
                        nc.vector.scalar_tensor_tensor(
                            aug[:, :, 10:11], rt, -1.0, aug[:, :, 9:10],
                            OP.mult, OP.subtract,
                        )
                        nc.vector.memset(aug[:, :, 11:13], -1.0)
                    else:
                        # [Yh Yl Yh | 1 1 | ryh ryl],  Y = 2y
                        nc.vector.tensor_scalar_mul(aug[:, :, 0:3], ch, 2.0)
                        nc.vector.scalar_tensor_tensor(
                            aug[:, :, 3:6], ch, 2.0, aug[:, :, 0:3],
                            OP.mult, OP.subtract,
                        )
                        nc.vector.tensor_copy(aug[:, :, 6:9], aug[:, :, 0:3])
                        nc.vector.memset(aug[:, :, 9:11], 1.0)
                        nc.vector.tensor_copy(aug[:, :, 11:12], rt)
                        nc.vector.tensor_sub(aug[:, :, 12:13], rt, aug[:, :, 11:12])

                    # one PSUM fill: transposes packed 4 q's per instruction
                    # ([128, 128] -> [128, 128]), then strided DVE evacuation
                    PK = 4  # q's packed per transpose
                    NG = NQ // PK
                    pt = psum_pool.tile([NP, NG * NP], BF16, tag="ps")
                    for g in range(NG):
                        nc.tensor.transpose(
                            pt[:, g * NP : (g + 1) * NP],
                            aug[:, g * PK : (g + 1) * PK, :].rearrange(
                                "p q c -> p (q c)"
                            ),
                            ident,
                        )
                    dst = chx[b] if side == "x" else chy[b]
                    # pt row 32*dq+c, col g*128+p  ->  chx[c, (g*PK+dq)*128+p]
                    for dq in range(PK):
                        nc.vector.tensor_copy(
                            dst.rearrange("c (g w p) -> c g w p", w=PK, p=NP)[
                                :, :, dq, :
                            ],
                            pt[32 * dq : 32 * dq + 13, :].rearrange(
                                "c (g p) -> c g p", p=NP
                            ),
                        )

            # ---- accumulators ----
            colacc = [singles.tile([NP, n], BF16, tag=f"colacc{b}",
                                   name=f"colacc{b}") for b in range(b_loc)]
            for b in range(b_loc):
                nc.vector.memset(colacc[b], NEG_BIG)
            rowtile = [singles.tile([NP, NQ], F32, tag=f"rowtile{b}",
                                    name=f"rowtile{b}") for b in range(b_loc)]

            # ---- main: PE fill -> ACT evac/cast -> DVE colmax TT + rowmax
            # fold-tree.  All DVE reduce ops are 1x mode, but TENSOR_TENSOR
            # max on bf16 SBUF runs 2x, so the per-tile rowmax is computed as
            # a pairwise-max fold (4096->2048->1024->512) followed by one
            # small 1x cache-reduce.  sb spans the whole row-tile so every
            # DVE/ACT op is as wide as possible (per-op overhead amortized).
            for b in range(b_loc):
                for r in range(NQ):
                    lhsT = chx[b][:, r * NP : (r + 1) * NP]
                    sb = evac.tile([NP, n], BF16, tag="sb")
                    for h in range(NH):
                        ps = psum_pool.tile([NP, HW], F32, tag="ps")
                        for s in range(HW // 512):
                            c0 = h * HW + s * 512
                            nc.tensor.matmul(
                                ps[:, s * 512 : (s + 1) * 512],
                                lhsT=lhsT,
                                rhs=chy[b][:, c0 : c0 + 512],
                                start=True,
                                stop=True,
                            )
                        nc.scalar.activation(
                            sb[:, h * HW : (h + 1) * HW], ps, ACTF.Copy
                        )
                    nc.vector.tensor_tensor(colacc[b], colacc[b], sb, op=OP.max)
                    u = smalls.tile([NP, n // 2], BF16, tag="u")
                    nc.vector.tensor_tensor(
                        u, sb[:, 0 : n // 2], sb[:, n // 2 : n], op=OP.max
                    )
                    v = smalls.tile([NP, n // 4], BF16, tag="v")
                    nc.vector.tensor_tensor(
                        v, u[:, 0 : n // 4], u[:, n // 4 : n // 2], op=OP.max
                    )
                    w = smalls.tile([NP, n // 8], BF16, tag="w")
                    nc.vector.tensor_tensor(
                        w, v[:, 0 : n // 8], v[:, n // 8 : n // 4], op=OP.max
                    )
                    junkw = smalls.tile([NP, n // 8], BF16, tag="junkw")
                    nc.vector.tensor_scalar(
                        out=junkw,
                        in0=w,
                        scalar1=NEG_BIG,
                        scalar2=None,
                        op0=OP.max,
                        op1=OP.max,
                        accum_out=rowtile[b][:, r : r + 1],
                    )

            # ---- finals ----
            sums = singles.tile([NP, 2 * b_loc], F32, tag="sums", name="sums")
            for b in range(b_loc):
                # row side: sum the per-tile rowmax partials
                nc.vector.tensor_reduce(sums[:, b : b + 1], rowtile[b],
                                        axis=AX.X, op=OP.add)
                # col side: transpose colacc (bf16), rowmax-reduce, sum
                pt2 = psum_pool.tile([NP, n], BF16, tag="ps")
                for t in range(NQ):
                    nc.tensor.transpose(
                        pt2[:, t * NP : (t + 1) * NP],
                        colacc[b][:, t * NP : (t + 1) * NP],
                        ident,
                    )
                cmax = smalls.tile([NP, NQ], F32, tag="cmax")
                nc.vector.tensor_reduce(
                    cmax, pt2.rearrange("p (t v) -> p t v", t=NQ),
                    axis=AX.X, op=OP.max,
                )
                nc.vector.tensor_reduce(sums[:, b_loc + b : b_loc + b + 1], cmax,
                                        axis=AX.X, op=OP.add)
            nc.sync.dma_start(out=out_d, in_=sums)

        if hw_loop:
            with tc.For_i(0, reps, 1, staggered_reset=staggered_reset):
                emit_body()
        else:
            for _ in range(reps):
                emit_body()

    nc.compile()
    return nc


_CACHE = {}


def _get_nc():
    if "nc" not in _CACHE:
        _CACHE["nc"] = build_nc(
            b_loc=B_FULL // N_CORES, n=N_FULL, c_in=C_FULL, num_devices=N_CORES
        )
    return _CACHE["nc"]


def kernel(x: np.ndarray, y: np.ndarray) -> np.ndarray:
    x = np.ascontiguousarray(np.asarray(x, dtype=np.float32))
    y = np.ascontiguousarray(np.asarray(y, dtype=np.float32))
    assert x.shape == (B_FULL, N_FULL, C_FULL), x.shape
    nc = _get_nc()
    bl = B_FULL // N_CORES
    in_maps = [
        {
            "x": np.ascontiguousarray(x[i * bl : (i + 1) * bl]),
            "y": np.ascontiguousarray(y[i * bl : (i + 1) * bl]),
        }
        for i in range(N_CORES)
    ]
    res = run_bass_kernel_spmd(nc, in_maps, list(range(N_CORES)))
    total = sum(float(r["partial"].astype(np.float64).sum()) for r in res.results)
    loss = -total / float(B_FULL * N_FULL)
    return np.float32(loss)


# revision 18
# speedup vs baseline: 151.1035x; 1.0003x over previous
"""ChamferLoss Trainium2 kernel.

Strategy (per core, data-parallel over batch: 16 batches / 8 cores = 2 each):
  pdist[b,i,j] = ||x_i||^2 + ||y_j||^2 - 2 x_i.y_j   (first 3 channels)
  loss = mean_bj(min_i pdist) + mean_bi(min_j pdist)

m = -pdist comes from a single K=13 bf16 augmented matmul (hi/lo split gives
fp32-class accuracy at bf16 PE speed):
  x-side rows: [xh(3), xh(3), xl(3), -rxh, -rxl, -1, -1]
  y-side rows: [Yh(3), Yl(3), Yh(3),  1,    1,  Ryh, Ryl],  Y = 2y, Ry=||y||^2
min -> max flip: rowmax via tensor_scalar+accum, colmax via tensor_tensor max.

Main loop is a 3-engine pipeline over half-row-tiles [128, 2048]:
  PE:  4 matmuls fill one PSUM half (4 banks, f32); two halves double-buffer
  ACT: evacuates PSUM f32 -> SBUF bf16 (activation Copy with dtype cast)
  DVE: rowmax via tensor_scalar (bf16 SBUF = 4x mode) + colmax via
       tensor_tensor max into a bf16 accumulator (2x mode)
This keeps all three engines ~busy; ACT evacuation (~1.9us/half-tile) is the
steady-state bottleneck => ~240us main loop + prep/finals.

Finals: per-tile rowmax halves get max-combined then summed; colacc gets
PE-transposed (bf16) and max/add-reduced. Output is per-partition partial
sums [128, 2*b_loc]; the host does the final 128-way gather-sum.

build_nc(reps=R, hw_loop=True) wraps the whole body in a tc.For_i hardware
loop: the NEFF size is then independent of R, which lets a timing harness
measure marginal device time per repetition without NEFF-size-dependent
host/dispatch overhead polluting the difference.
"""

from contextlib import ExitStack

import numpy as np

import concourse.bass as bass
import concourse.bacc as bacc
import concourse.tile as tile
from concourse import bass_isa, mybir
from concourse.bass_utils import run_bass_kernel_spmd
from concourse.masks import make_identity

F32 = mybir.dt.float32
BF16 = mybir.dt.bfloat16
AX = mybir.AxisListType
OP = mybir.AluOpType
ACTF = mybir.ActivationFunctionType

NEG_BIG = -3.0e38

B_FULL = 16
N_FULL = 4096
C_FULL = 6
N_CORES = 8


def build_nc(b_loc=2, n=4096, c_in=6, num_devices=8, reps=1, hw_loop=False,
             staggered_reset=True):
    """Per-core program. Inputs x,y: [b_loc, n, c_in] f32; output "partial"
    [128, 2*b_loc] f32 per-partition partial sums of rowmax/colmax of -pdist."""
    NP = 128
    NQ = n // NP                  # row-tiles per batch (32)
    NH = 2                        # half-tiles per row-tile
    HW = n // NH                  # half-tile width (2048)

    nc = bacc.Bacc(
        "TRN2",
        target_bir_lowering=False,
        debug=False,
        enable_asserts=False,
        num_devices=num_devices,
    )

    x_d = nc.declare_dram_parameter("x", [b_loc, n, c_in], F32, isOutput=False).ap()
    y_d = nc.declare_dram_parameter("y", [b_loc, n, c_in], F32, isOutput=False).ap()
    out_d = nc.declare_dram_parameter(
        "partial", [NP, 2 * b_loc], F32, isOutput=True
    ).ap()

    with tile.TileContext(nc) as tc, ExitStack() as ctx:
        prep = ctx.enter_context(tc.tile_pool(name="prep", bufs=2))
        singles = ctx.enter_context(tc.tile_pool(name="singles", bufs=1))
        psum_pool = ctx.enter_context(tc.tile_pool(name="psum", bufs=2, space="PSUM"))
        evac = ctx.enter_context(tc.tile_pool(name="evac", bufs=3))
        smalls = ctx.enter_context(tc.tile_pool(name="smalls", bufs=2))

        ident = singles.tile([NP, NP], BF16, tag="ident", name="ident")
        make_identity(nc, ident)

        def emit_body():
            chx = [singles.tile([13, n], BF16, tag=f"chx{b}", name=f"chx{b}")
                   for b in range(b_loc)]
            chy = [singles.tile([13, n], BF16, tag=f"chy{b}", name=f"chy{b}")
                   for b in range(b_loc)]

            # ---- prep: aug point-major (DVE only), PE-transpose, DVE evac
            for b in range(b_loc):
                for side in ("x", "y"):
                    src = x_d if side == "x" else y_d
                    xin = prep.tile([NP, NQ, c_in], F32, tag="xin")
                    nc.sync.dma_start(
                        out=xin, in_=src[b].rearrange("(p q) c -> p q c", p=NP)
                    )
                    # channel dim padded 13 -> 32 so 4 q's pack into one
                    # [128,128] transpose with 32-aligned output partitions
                    aug = prep.tile([NP, NQ, 32], BF16, tag="aug")
                    sq = prep.tile([NP, NQ, 3], F32, tag="sq")
                    rt = prep.tile([NP, NQ, 1], F32, tag="rt")
                    ch = xin[:, :, 0:3]
                    nc.gpsimd.memset(aug[:, :, 13:32], 0.0)
                    nc.vector.tensor_mul(sq, ch, ch)
                    nc.vector.tensor_reduce(rt, sq, axis=AX.X, op=OP.add)
                    if side == "x":
                        # [xh xh xl | -rxh -rxl | -1 -1]
                        nc.vector.tensor_copy(aug[:, :, 0:3], ch)
                        nc.vector.tensor_copy(aug[:, :, 3:6], aug[:, :, 0:3])
                        nc.vector.tensor_sub(aug[:, :, 6:9], ch, aug[:, :, 0:3])
                        nc.vector.tensor_scalar_mul(aug[:, :, 9:10], rt, -1.0)
                        nc.vector.scalar_tensor_tensor(
                            aug[:, :, 10:11], rt, -1.0, aug[:, :, 9:10],
                            OP.mult, OP.subtract,
                        )
                        nc.vector.memset(aug[:, :, 11:13], -1.0)
                    else:
                        # [Yh Yl Yh | 1 1 | ryh ryl],  Y = 2y
                        nc.vector.tensor_scalar_mul(aug[:, :, 0:3], ch, 2.0)
                        nc.vector.scalar_tensor_tensor(
                            aug[:, :, 3:6], ch, 2.0, aug[:, :, 0:3],
                            OP.mult, OP.subtract,
                        )
                        nc.vector.tensor_copy(aug[:, :, 6:9], aug[:, :, 0:3])
                        nc.vector.memset(aug[:, :, 9:11], 1.0)
                        nc.vector.tensor_copy(aug[:, :, 11:12], rt)
                        nc.vector.tensor_sub(aug[:, :, 12:13], rt, aug[:, :, 11:12])

                    # one PSUM fill: transposes packed 4 q's per instruction
                    # ([128, 128] -> [128, 128]), then strided DVE evacuation
                    PK = 4  # q's packed per transpose
                    NG = NQ // PK
                    pt = psum_pool.tile([NP, NG * NP], BF16, tag="ps")
                    for g in range(NG):
                        nc.tensor.transpose(
                            pt[:, g * NP : (g + 1) * NP],
                            aug[:, g * PK : (g + 1) * PK, :].rearrange(
                                "p q c -> p (q c)"
                            ),
                            ident,
                        )
                    dst = chx[b] if side == "x" else chy[b]
                    # pt row 32*dq+c, col g*128+p  ->  chx[c, (g*PK+dq)*128+p]
                    # evacuation on ScalarE: DVE is the kernel's critical
                    # engine, ACT has slack
                    for dq in range(PK):
                        nc.scalar.activation(
                            dst.rearrange("c (g w p) -> c g w p", w=PK, p=NP)[
                                :, :, dq, :
                            ],
                            pt[32 * dq : 32 * dq + 13, :].rearrange(
                                "c (g p) -> c g p", p=NP
                            ),
                            ACTF.Copy,
                        )

            # ---- accumulators ----
            colacc = [singles.tile([NP, n], BF16, tag=f"colacc{b}",
                                   name=f"colacc{b}") for b in range(b_loc)]
            for b in range(b_loc):
                nc.gpsimd.memset(colacc[b], NEG_BIG)
            rowtile = [singles.tile([NP, NQ], F32, tag=f"rowtile{b}",
                                    name=f"rowtile{b}") for b in range(b_loc)]

            # ---- main: PE fill -> ACT evac/cast -> DVE colmax TT + rowmax
            # fold-tree.  All DVE reduce ops are 1x mode, but TENSOR_TENSOR
            # max on bf16 SBUF runs 2x, so the per-tile rowmax is computed as
            # a pairwise-max fold (4096->2048->1024->512) followed by one
            # small 1x cache-reduce.  sb spans the whole row-tile so every
            # DVE/ACT op is as wide as possible (per-op overhead amortized).
            for b in range(b_loc):
                wpair = None
                for r in range(NQ):
                    lhsT = chx[b][:, r * NP : (r + 1) * NP]
                    sb = evac.tile([NP, n], BF16, tag="sb")
                    for h in range(NH):
                        ps = psum_pool.tile([NP, HW], F32, tag="ps")
                        for s in range(HW // 512):
                            c0 = h * HW + s * 512
                            nc.tensor.matmul(
                                ps[:, s * 512 : (s + 1) * 512],
                                lhsT=lhsT,
                                rhs=chy[b][:, c0 : c0 + 512],
                                start=True,
                                stop=True,
                            )
                        nc.scalar.activation(
                            sb[:, h * HW : (h + 1) * HW], ps, ACTF.Copy
                        )
                    nc.vector.tensor_tensor(colacc[b], colacc[b], sb, op=OP.max)
                    u = smalls.tile([NP, n // 2], BF16, tag="u")
                    nc.vector.tensor_tensor(
                        u, sb[:, 0 : n // 2], sb[:, n // 2 : n], op=OP.max
                    )
                    v = smalls.tile([NP, n // 4], BF16, tag="v")
                    nc.vector.tensor_tensor(
                        v, u[:, 0 : n // 4], u[:, n // 4 : n // 2], op=OP.max
                    )
                    if r % 2 == 0:
                        wpair = smalls.tile([NP, 2, n // 8], BF16, tag="wpair")
                    nc.vector.tensor_tensor(
                        wpair[:, r % 2, :],
                        v[:, 0 : n // 8], v[:, n // 8 : n // 4], op=OP.max
                    )
                    if r % 2 == 1:
                        nc.vector.tensor_reduce(
                            rowtile[b][:, r - 1 : r + 1], wpair,
                            axis=AX.X, op=OP.max,
                        )

            # ---- finals ----
            sums = singles.tile([NP, 2 * b_loc], F32, tag="sums", name="sums")
            for b in range(b_loc):
                # row side: sum the per-tile rowmax partials
                nc.vector.tensor_reduce(sums[:, b : b + 1], rowtile[b],
                                        axis=AX.X, op=OP.add)
                # col side: transpose colacc (bf16), rowmax-reduce, sum
                pt2 = psum_pool.tile([NP, n], BF16, tag="ps")
                for t in range(NQ):
                    nc.tensor.transpose(
                        pt2[:, t * NP : (t + 1) * NP],
                        colacc[b][:, t * NP : (t + 1) * NP],
                        ident,
                    )
                cmax = smalls.tile([NP, NQ], F32, tag="cmax")
                nc.vector.tensor_reduce(
                    cmax, pt2.rearrange("p (t v) -> p t v", t=NQ),
                    axis=AX.X, op=OP.max,
                )
                nc.vector.tensor_reduce(sums[:, b_loc + b : b_loc + b + 1], cmax,
                                        axis=AX.X, op=OP.add)
            nc.sync.dma_start(out=out_d, in_=sums)

        if hw_loop:
            with tc.For_i(0, reps, 1, staggered_reset=staggered_reset):
                emit_body()
        else:
            for _ in range(reps):
                emit_body()

    nc.compile()
    return nc


_CACHE = {}


def _get_nc():
    if "nc" not in _CACHE:
        _CACHE["nc"] = build_nc(
            b_loc=B_FULL // N_CORES, n=N_FULL, c_in=C_FULL, num_devices=N_CORES
        )
    return _CACHE["nc"]


def kernel(x: np.ndarray, y: np.ndarray) -> np.ndarray:
    x = np.ascontiguousarray(np.asarray(x, dtype=np.float32))
    y = np.ascontiguousarray(np.asarray(y, dtype=np.float32))
    assert x.shape == (B_FULL, N_FULL, C_FULL), x.shape
    nc = _get_nc()
    bl = B_FULL // N_CORES
    in_maps = [
        {
            "x": np.ascontiguousarray(x[i * bl : (i + 1) * bl]),
            "y": np.ascontiguousarray(y[i * bl : (i + 1) * bl]),
        }
        for i in range(N_CORES)
    ]
    res = run_bass_kernel_spmd(nc, in_maps, list(range(N_CORES)))
    total = sum(float(r["partial"].astype(np.float64).sum()) for r in res.results)
    loss = -total / float(B_FULL * N_FULL)
    return np.float32(loss)
